# revision 4
# baseline (speedup 1.0000x reference)
"""Trainium2 Bass kernel for nn_DefaultSegmentLinear.

Computes out[M, N] = (x[M, K] @ W[N, K]^T) * (s_x * s_w[chunk]) + bias[N]
with M=8192, K=4096, N=4096 (C=4 chunks of 1024 out-features).

Strategy
--------
- Host: fold the per-chunk scales into W, transpose x and W to put the
  contraction dim (K) on partitions, and cast both to bf16 (the PE
  upcasts bf16 to FP22 internally and accumulates fp32 in PSUM;
  measured rel err ~1.5e-3 vs the 2e-2 gate). bf16 halves HBM traffic
  and SBUF footprint vs fp32/fp32r at the same PE rate (the cost model
  charges 1 cycle/row for bf16 and fp32r alike; moving-row count, not
  dtype, sets the ~437us/core PE floor at 2.4 GHz).
- Sharding: M sharded 8 ways (tokens). Each core holds its x^T slice
  [K, 1024] resident in SBUF (8 MiB bf16), streams the full W^T
  [K, 4096] once (32 MiB bf16), and writes out^T [4096, 1024] fp32
  once (16 MiB): 56 MiB of HBM traffic per core, hidden under the PE
  floor (measured pure-DMA rate ~470 GB/s/core).
- Device loop: out^T is produced in 16 n-blocks of 256 columns. Each
  block accumulates 2 n-subtiles x 2 m-halves in 4 PSUM banks over all
  32 k-tiles, so consecutive blocks alternate PSUM bank sets 0-3/4-7
  and the first matmul of block nb+1 never waits on block nb's drains.
- Engine-queue roles (in-order queues; only SP/ACT/gpsimd issue DMAs):
  SP carries the W stream (2 MiB 16-k-tile groups, 4-buffer prefetch)
  plus the x/bias loads, interleaved x0-3, W(b0,g0), x4-15, W(b0,g1),
  x16-31 so a fresh pass starts its first W transfer after only 4 x
  reloads; ACT carries the PSUM drains (bias-add fused, per-partition
  scalars) and the out DMAs that depend on them. Keeping drains and W
  triggers on separate queues removes the block-boundary serialization
  that cost the fp32r baseline ~8-10%.
- Output is produced transposed ([N, M] per core); the host
  concatenates the 8 core slices and transposes back.
"""

import numpy as np
import ml_dtypes

import concourse.bacc as bacc
import concourse.mybir as mybir
import concourse.tile as tile
from concourse import bass_utils

P = 128
M, K, N = 8192, 4096, 4096
N_CORES = 8
MC = M // N_CORES           # 1024 rows of x per core
KT = K // P                 # 32 k-tiles
NB = 256                    # n-block width (2 psum banks x 2 m-halves)
NBLK = N // NB              # 16 n-blocks
NSUB = NB // P              # 2 n-subtiles per block
MHW = 512                   # moving free dim per matmul (psum bank cap)
MH = MC // MHW              # 2 m-halves
KG = 16                     # k-tiles per W-group DMA
NGRP = KT // KG             # 2 groups per n-block

F32 = mybir.dt.float32
BF16 = mybir.dt.bfloat16

_CACHE: dict = {}


def _build(iters: int = 1):
    """Build + compile the per-core Bass program.

    iters > 1 wraps the body in a hardware loop (for timing runs).
    """
    nc = bacc.Bacc("TRN2", target_bir_lowering=False, debug=False)
    xT_d = nc.dram_tensor("xT", [K, MC], BF16, kind="ExternalInput").ap()
    wT_d = nc.dram_tensor("wT", [K, N], BF16, kind="ExternalInput").ap()
    # bias pre-arranged host-side as [128, N/128]: column j holds
    # bias[j*128 : (j+1)*128] (per-partition scalars for the ACT drain).
    bias_d = nc.dram_tensor("biasc", [P, N // P], F32, kind="ExternalInput").ap()
    outT_d = nc.dram_tensor("outT", [N, MC], F32, kind="ExternalOutput").ap()

    with tile.TileContext(nc) as tc:
        with (
            tc.tile_pool(name="xres", bufs=KT) as xres_pool,
            tc.tile_pool(name="wstream", bufs=4) as w_pool,
            tc.tile_pool(name="biasp", bufs=1) as bias_pool,
            tc.tile_pool(name="ostage", bufs=8) as o_pool,
            tc.tile_pool(name="psum", bufs=8, space="PSUM") as psum_pool,
        ):
            def body(it):
                bias_sb = bias_pool.tile([P, N // P], F32, name="biassb")
                nc.sync.dma_start(bias_sb[:], bias_d[:])
                x_res = [xres_pool.tile([P, MC], BF16, tag="x",
                                        name=f"x{it}_{kt}")
                         for kt in range(KT)]

                def x_load(kts):
                    for kt in kts:
                        nc.sync.dma_start(
                            x_res[kt][:], xT_d[kt * P:(kt + 1) * P, :])

                w_tiles = {}

                def w_load(nb, g):
                    w_g = w_pool.tile([P, KG, NB], BF16, tag="w",
                                      name=f"w{it}_{nb}_{g}")
                    nc.sync.dma_start(
                        w_g[:],
                        wT_d[g * KG * P:(g + 1) * KG * P,
                             nb * NB:(nb + 1) * NB].rearrange(
                                 "(kg p) n -> p kg n", p=P))
                    w_tiles[(nb, g)] = w_g

                x_load(range(0, 4))
                w_load(0, 0)
                x_load(range(4, 16))
                w_load(0, 1)
                x_load(range(16, KT))

                for nb in range(NBLK):
                    if nb + 1 < NBLK:
                        for g in range(NGRP):
                            w_load(nb + 1, g)
                    psums = [
                        [psum_pool.tile([P, MHW], F32, tag="ps",
                                        name=f"ps{it}_{nb}_{nt}_{mh}")
                         for mh in range(MH)]
                        for nt in range(NSUB)
                    ]
                    for g in range(NGRP):
                        w_g = w_tiles.pop((nb, g))
                        for ki in range(KG):
                            kt = g * KG + ki
                            for nt in range(NSUB):
                                for mh in range(MH):
                                    nc.tensor.matmul(
                                        psums[nt][mh][:],
                                        w_g[:, ki, nt * P:(nt + 1) * P],
                                        x_res[kt][:, mh * MHW:(mh + 1) * MHW],
                                        start=(kt == 0),
                                        stop=(kt == KT - 1),
                                    )
                    for nt in range(NSUB):
                        ncol = nb * NSUB + nt
                        for mh in range(MH):
                            o_sb = o_pool.tile([P, MHW], F32, tag="o",
                                               name=f"o{it}_{nb}_{nt}_{mh}")
                            nc.scalar.activation(
                                out=o_sb[:], in_=psums[nt][mh][:],
                                func=mybir.ActivationFunctionType.Identity,
                                bias=bias_sb[:, ncol:ncol + 1],
                            )
                            nc.scalar.dma_start(
                                outT_d[ncol * P:(ncol + 1) * P,
                                       mh * MHW:(mh + 1) * MHW],
                                o_sb[:])

            if iters == 1:
                body(0)
            else:
                # For_i runs an InstAllEngineBarrier in its per-iteration
                # semaphore-reset block; unroll 4 bodies per iteration and
                # use staggered per-stage resets (one stage per body) so
                # engines never globally drain between timing iterations.
                if iters % 4 == 0:
                    with tc.For_i(0, iters // 4, 1, staggered_reset=True):
                        for u in range(4):
                            if u:
                                tc.stage_boundary()
                            body(u)
                else:
                    with tc.For_i(0, iters, 1):
                        body(0)
    nc.compile()
    return nc


def _prep_inputs(x, w_chunks, bias, input_scale, weight_scales):
    s = (np.float32(input_scale[0]) * weight_scales.astype(np.float32))
    W = w_chunks.reshape(N, K).astype(np.float32)
    W = W * np.repeat(s, N // s.shape[0]).astype(np.float32)[:, None]
    WTb = W.T.astype(ml_dtypes.bfloat16)                     # [K, N]
    xTb = x.astype(np.float32).T.astype(ml_dtypes.bfloat16)  # [K, M]
    bias_c = np.ascontiguousarray(
        bias.astype(np.float32).reshape(N // P, P).T)        # [128, N/128]
    in_maps = []
    for c in range(N_CORES):
        in_maps.append({
            "xT": np.ascontiguousarray(xTb[:, c * MC:(c + 1) * MC]),
            "wT": WTb,
            "biasc": bias_c,
        })
    return in_maps


def kernel(x, w_chunks, bias, input_scale, weight_scales):
    x = np.asarray(x)
    w_chunks = np.asarray(w_chunks)
    bias = np.asarray(bias)
    input_scale = np.asarray(input_scale)
    weight_scales = np.asarray(weight_scales)
    if "nc" not in _CACHE:
        _CACHE["nc"] = _build(iters=1)
    nc = _CACHE["nc"]
    in_maps = _prep_inputs(x, w_chunks, bias, input_scale, weight_scales)
    res = bass_utils.run_bass_kernel_spmd(
        nc, in_maps, core_ids=list(range(N_CORES)))
    outT = np.concatenate(
        [res.results[c]["outT"] for c in range(N_CORES)], axis=1)  # [N, M]
    return np.ascontiguousarray(outT.T)
